# revision 1
# baseline (speedup 1.0000x reference)
"""Trainium2 Bass kernel for a debiased GRU cell.

Computation (per batch row):
    r   = sigmoid(W_r @ [x; h] + b_r)
    u   = sigmoid(W_u @ [x; h] + b_u)
    hh  = tanh(W_h @ [x_int; r*h] + b_h)
    s   = score * u
    out = (1 - s) * hh + s * h

Strategy: data-parallel over 8 cores (8192 rows each). On-chip layout is
feature-major ([H, batch]) so that
  - activations never need an on-chip transpose (host supplies x.T / h.T),
  - gate biases fuse into the ACT engine's per-partition bias operand,
  - matmuls run with full K=128 / M=128 / N=512 tiles (PE at peak rate).
The only broadcast needed (attention score along partitions) runs on the
otherwise-idle GPSIMD engine. Output is produced as out.T and un-transposed
on the host.
"""

import os

import numpy as np

import concourse.bacc as bacc
import concourse.bass as bass
import concourse.mybir as mybir
import concourse.tile as tile
from concourse.bass_utils import run_bass_kernel_spmd

B = 65536
I = 256
H = 256
NCORES = 8
BC = B // NCORES  # rows per core
NB = 512          # batch columns per block (max fp32 matmul free dim)
NBLK = BC // NB   # 16
FP32 = mybir.dt.float32
AF = mybir.ActivationFunctionType

_NC_CACHE = {}


def _build_nc(reps=1, loop=None, mm_dtype="fp32r",
              pg_bufs=6, ph_bufs=2, in_bufs=4, work_bufs=3, psum_fine=True,
              out_queue="scalar", split_loads=True, group=1):
    nc = bacc.Bacc(
        "TRN2",
        target_bir_lowering=False,
        debug=False,
        enable_asserts=False,
    )

    # Matmul-operand dtype. float32r streams fp32 bits through the PE at
    # full rate (1 cycle/row vs 4 for plain fp32); walrus requires every
    # producer of an fp32r-matmul operand to declare an fp32r output, so
    # the whole feeding path (DRAM tensor -> DMA -> SBUF tile -> matmul)
    # is declared float32r. Bit layout is identical to fp32.
    MDT = {"fp32": mybir.dt.float32, "fp32r": mybir.dt.float32r}[mm_dtype]

    xT = nc.dram_tensor("xT", [2 * I, BC], MDT, kind="ExternalInput")
    hT = nc.dram_tensor("hT", [H, BC], MDT, kind="ExternalInput")
    sc = nc.dram_tensor("sc", [NBLK, 1, NB], FP32, kind="ExternalInput")
    wg = nc.dram_tensor("wg", [128, 24 * 128], MDT, kind="ExternalInput")
    wh = nc.dram_tensor("wh", [128, 8 * 128], MDT, kind="ExternalInput")
    bg = nc.dram_tensor("bg", [128, 4], FP32, kind="ExternalInput")
    bh = nc.dram_tensor("bh", [128, 2], FP32, kind="ExternalInput")
    outT = nc.dram_tensor("outT", [H, BC], FP32, kind="ExternalOutput")

    # [blk, partition, k-chunk, col] — DMA at `group`-block granularity
    GNB = group * NB
    xTr = xT.rearrange("(k p) (b n) -> b p k n", p=128, n=GNB)
    hTr = hT.rearrange("(k p) (b n) -> b p k n", p=128, n=GNB)
    scr = sc.rearrange("b o n -> b o n") if group == 1 else \
        sc.rearrange("(g j) o n -> g o (j n)", j=group)
    outTr = outT.rearrange("(m p) (b n) -> b p m n", p=128, n=GNB)

    with tile.TileContext(nc) as tc:
        with (
            tc.tile_pool(name="const", bufs=1) as cpool,
            tc.tile_pool(name="xin", bufs=in_bufs) as xpool,
            tc.tile_pool(name="hin", bufs=in_bufs) as hpool,
            tc.tile_pool(name="sin", bufs=in_bufs) as spool,
            tc.tile_pool(name="gates", bufs=work_bufs) as gpool,
            tc.tile_pool(name="work", bufs=work_bufs) as wpool,
            tc.tile_pool(name="outp", bufs=work_bufs) as opool,
            tc.tile_pool(name="psg", bufs=pg_bufs, space=bass.MemorySpace.PSUM) as pgpool,
            tc.tile_pool(name="psh", bufs=ph_bufs, space=bass.MemorySpace.PSUM) as phpool,
        ):
            # Gate weights split per gate-half so the first gate chain only
            # waits on its own 300 KB slice, not the full 2.1 MB weight load.
            wg_sb = cpool.tile([128, 24 * 128], MDT)
            for gi in range(4):
                nc.sync.dma_start(wg_sb[:, gi * 768:(gi + 1) * 768],
                                  wg[:, gi * 768:(gi + 1) * 768])
            bg_sb = cpool.tile([128, 4], FP32)
            nc.sync.dma_start(bg_sb[:], bg[:])
            wh_sb = cpool.tile([128, 8 * 128], MDT)
            nc.sync.dma_start(wh_sb[:], wh[:])
            bh_sb = cpool.tile([128, 2], FP32)
            nc.sync.dma_start(bh_sb[:], bh[:])

            def load_group(g):
                """DMA the inputs for blocks [g*group, (g+1)*group) in one
                burst each, plus the group-wide output staging tile."""
                xt = xpool.tile([128, 4, GNB], MDT, tag="xt")
                if split_loads:
                    # two half-loads: the gate chains only wait on the half
                    # they read next, hiding more DMA latency inside a block
                    nc.sync.dma_start(xt[:, 0:2, :], xTr[g][:, 0:2, :])
                    nc.sync.dma_start(xt[:, 2:4, :], xTr[g][:, 2:4, :])
                else:
                    nc.sync.dma_start(xt[:], xTr[g])
                ht = hpool.tile([128, 2, GNB], MDT, tag="ht")
                nc.sync.dma_start(ht[:], hTr[g])
                srow = spool.tile([1, GNB], FP32, tag="srow")
                nc.sync.dma_start(srow[:], scr[g])
                sbc = spool.tile([128, 2, GNB], FP32, tag="sbc")
                nc.gpsimd.partition_broadcast(sbc[:, 0, :], srow[:])
                nc.gpsimd.partition_broadcast(sbc[:, 1, :], srow[:])
                og = opool.tile([128, 2, GNB], FP32, tag="o")
                return dict(g=g, xt=xt, ht=ht, sbc=sbc, og=og)

            def emit_gates(grp, j):
                """Gate matmuls + sigmoids + r*h for sub-block j of a group."""
                b = grp["g"] * group + j
                js = slice(j * NB, (j + 1) * NB)
                xt = grp["xt"][:, :, js]
                ht = grp["ht"][:, :, js]

                if psum_fine:
                    pgs = [pgpool.tile([128, NB], FP32, tag="pg", name=f"pg{b}_{i}") for i in range(4)]
                else:
                    pg_r = pgpool.tile([128, 2, NB], FP32, tag="pg")
                    pg_u = pgpool.tile([128, 2, NB], FP32, tag="pg")
                    pgs = [pg_r[:, 0, :], pg_r[:, 1, :], pg_u[:, 0, :], pg_u[:, 1, :]]
                for gi in range(4):  # r0, r1, u0, u1
                    dst = pgs[gi][:] if psum_fine else pgs[gi]
                    for k in range(6):
                        act = xt[:, k, :] if k < 4 else ht[:, k - 4, :]
                        c = gi * 6 + k
                        nc.tensor.matmul(
                            dst,
                            wg_sb[:, c * 128:(c + 1) * 128],
                            act,
                            start=(k == 0),
                            stop=(k == 5),
                        )
                r = gpool.tile([128, 2, NB], FP32, tag="r")
                u = gpool.tile([128, 2, NB], FP32, tag="u")
                for m in range(2):
                    nc.scalar.activation(
                        r[:, m, :], pgs[m][:] if psum_fine else pgs[m],
                        AF.Sigmoid, bias=bg_sb[:, m:m + 1]
                    )
                    nc.scalar.activation(
                        u[:, m, :], pgs[2 + m][:] if psum_fine else pgs[2 + m],
                        AF.Sigmoid, bias=bg_sb[:, 2 + m:3 + m]
                    )
                rh = wpool.tile([128, 2, NB], MDT, tag="rh")
                nc.vector.tensor_mul(rh[:], r[:], ht)
                # e2 = score*u and A = h*e2 only depend on the gate phase, so
                # they run here, off the post-tanh critical tail.
                e2 = wpool.tile([128, 2, NB], FP32, tag="e2")
                nc.vector.tensor_mul(e2[:], u[:], grp["sbc"][:, :, js])
                A = wpool.tile([128, 2, NB], FP32, tag="A")
                nc.vector.tensor_mul(A[:], ht, e2[:])
                return dict(b=b, j=j, grp=grp, xt=xt, rh=rh, e2=e2, A=A)

            def emit_h(st):
                """h_hat matmul + tanh + final combine + store for block b."""
                b = st["b"]
                if psum_fine:
                    phs = [phpool.tile([128, NB], FP32, tag="ph", name=f"ph{b}_{i}") for i in range(2)]
                else:
                    ph = phpool.tile([128, 2, NB], FP32, tag="ph")
                    phs = [ph[:, 0, :], ph[:, 1, :]]
                for m in range(2):
                    for k in range(4):
                        act = st["xt"][:, k] if k < 2 else st["rh"][:, k - 2, :]
                        c = m * 4 + k
                        nc.tensor.matmul(
                            phs[m][:] if psum_fine else phs[m],
                            wh_sb[:, c * 128:(c + 1) * 128],
                            act,
                            start=(k == 0),
                            stop=(k == 3),
                        )
                hhat = wpool.tile([128, 2, NB], FP32, tag="hhat")
                for m in range(2):
                    nc.scalar.activation(
                        hhat[:, m, :], phs[m][:] if psum_fine else phs[m],
                        AF.Tanh, bias=bh_sb[:, m:m + 1]
                    )
                # out = A - (e2-1)*hh  ==  hh + e2*(h - hh), with A = h*e2
                C = wpool.tile([128, 2, NB], FP32, tag="C")
                nc.vector.scalar_tensor_tensor(
                    C[:], st["e2"][:], 1.0, hhat[:],
                    op0=mybir.AluOpType.subtract, op1=mybir.AluOpType.mult,
                )
                j = st["j"]
                og = st["grp"]["og"]
                nc.vector.tensor_sub(og[:, :, j * NB:(j + 1) * NB],
                                     st["A"][:], C[:])
                if j == group - 1:
                    # store on the ACT HWDGE ring so it doesn't queue behind
                    # the input loads on the SP ring
                    out_eng = nc.scalar if out_queue == "scalar" else nc.sync
                    out_eng.dma_start(outTr[st["grp"]["g"]], og[:])

            # Software-pipelined emission: block b's h-chain is emitted after
            # block b+1's gate matmuls so the PE never waits on the r*h
            # elementwise product. reps>1 repeats the whole pass (same
            # output) — used only for slope-based timing in bench.py.
            def emit_pass():
                prev = None
                for _rep in range(reps):
                    for g in range(NBLK // group):
                        grp = load_group(g)
                        for j in range(group):
                            st = emit_gates(grp, j)
                            if prev is not None:
                                emit_h(prev)
                            prev = st
                emit_h(prev)

            if loop is None:
                emit_pass()
            else:
                # bench-only: repeat the whole pass `loop` times inside one
                # NEFF execution for slope-based timing.
                with tc.For_i(0, loop, 1):
                    emit_pass()

    nc.compile()
    return nc


def _get_nc():
    if "nc" not in _NC_CACHE:
        _NC_CACHE["nc"] = _build_nc()
    return _NC_CACHE["nc"]


def _pack_weights(W_r, W_u, W_h, b_r, b_u, b_h):
    wg = np.empty((128, 24 * 128), np.float32)
    for gi in range(4):
        W = W_r if gi < 2 else W_u
        m = gi % 2
        for k in range(6):
            c = gi * 6 + k
            wg[:, c * 128:(c + 1) * 128] = W[m * 128:(m + 1) * 128,
                                             k * 128:(k + 1) * 128].T
    wh = np.empty((128, 8 * 128), np.float32)
    for m in range(2):
        for k in range(4):
            c = m * 4 + k
            wh[:, c * 128:(c + 1) * 128] = W_h[m * 128:(m + 1) * 128,
                                               k * 128:(k + 1) * 128].T
    bg = np.stack([b_r[:128], b_r[128:], b_u[:128], b_u[128:]], axis=1)
    bh = np.stack([b_h[:128], b_h[128:]], axis=1)
    return (np.ascontiguousarray(wg), np.ascontiguousarray(wh),
            np.ascontiguousarray(bg), np.ascontiguousarray(bh))


def _make_in_maps(inputs, h_prev, attention_score, W_r, b_r, W_u, b_u, W_h, b_h):
    inputs = np.asarray(inputs, np.float32)
    h_prev = np.asarray(h_prev, np.float32)
    attention_score = np.asarray(attention_score, np.float32)
    wg, wh, bg, bh = _pack_weights(
        np.asarray(W_r, np.float32), np.asarray(W_u, np.float32),
        np.asarray(W_h, np.float32), np.asarray(b_r, np.float32),
        np.asarray(b_u, np.float32), np.asarray(b_h, np.float32),
    )
    in_maps = []
    for c in range(NCORES):
        sl = slice(c * BC, (c + 1) * BC)
        in_maps.append({
            "xT": np.ascontiguousarray(inputs[sl].T),
            "hT": np.ascontiguousarray(h_prev[sl].T),
            "sc": np.ascontiguousarray(attention_score[sl].reshape(NBLK, 1, NB)),
            "wg": wg, "wh": wh, "bg": bg, "bh": bh,
        })
    return in_maps


def _run(in_maps, trace=False, **kwargs):
    try:
        return run_bass_kernel_spmd(
            _get_nc(), in_maps, core_ids=list(range(NCORES)), trace=trace, **kwargs
        )
    except ModuleNotFoundError:
        # A global BASS_TRACE=1 enables the NTFF trace path, which needs
        # antenv.axon_hooks; on images without it, retry untraced. The env
        # override is scoped and restored so other users of the process are
        # unaffected.
        had = os.environ.get("BASS_NEVER_TRACE")
        os.environ["BASS_NEVER_TRACE"] = "1"
        try:
            return run_bass_kernel_spmd(
                _get_nc(), in_maps, core_ids=list(range(NCORES)), trace=False,
                **kwargs
            )
        finally:
            if had is None:
                del os.environ["BASS_NEVER_TRACE"]
            else:
                os.environ["BASS_NEVER_TRACE"] = had


def _gather(results):
    out = np.empty((B, H), np.float32)
    for c in range(NCORES):
        out[c * BC:(c + 1) * BC] = results[c]["outT"].T
    return out


def kernel(**inputs):
    res = _run(_make_in_maps(**inputs), trace=False)
    return _gather(res.results)



# revision 7
# speedup vs baseline: 1.0117x; 1.0117x over previous
"""Trainium2 Bass kernel for a debiased GRU cell.

Computation (per batch row):
    r   = sigmoid(W_r @ [x; h] + b_r)
    u   = sigmoid(W_u @ [x; h] + b_u)
    hh  = tanh(W_h @ [x_int; r*h] + b_h)
    s   = score * u
    out = (1 - s) * hh + s * h

Strategy: data-parallel over 8 cores (8192 rows each). On-chip layout is
feature-major ([H, batch]) so that
  - activations never need an on-chip transpose (host supplies x.T / h.T),
  - gate biases fuse into the ACT engine's per-partition bias operand,
  - matmuls run with full K=128 / M=128 / N=512 tiles (PE at peak rate).
The only broadcast needed (attention score along partitions) runs on the
otherwise-idle GPSIMD engine. Output is produced as out.T and un-transposed
on the host.
"""

import os

import ml_dtypes
import numpy as np

import concourse.bacc as bacc
import concourse.bass as bass
import concourse.mybir as mybir
import concourse.tile as tile
from concourse.bass_utils import run_bass_kernel_spmd

B = 65536
I = 256
H = 256
NCORES = 8
BC = B // NCORES  # rows per core
NB = 512          # batch columns per block (max fp32 matmul free dim)
NBLK = BC // NB   # 16
FP32 = mybir.dt.float32
AF = mybir.ActivationFunctionType

# Host-side dtype for matmul operands; must match _build_nc's mm_dtype.
MM_DTYPE = "bf16"
_HOST_MDT = {"fp32": np.float32, "fp32r": np.float32,
             "bf16": ml_dtypes.bfloat16}

_NC_CACHE = {}


def _build_nc(reps=1, loop=None, mm_dtype="bf16",
              pg_bufs=6, ph_bufs=2, in_bufs=4, work_bufs=3, psum_fine=True,
              out_queue="scalar", split_loads=True, group=1):
    nc = bacc.Bacc(
        "TRN2",
        target_bir_lowering=False,
        debug=False,
        enable_asserts=False,
    )

    # Matmul-operand dtype. float32r streams fp32 bits through the PE at
    # full rate (1 cycle/row vs 4 for plain fp32); bf16 runs at the same
    # PE rate but halves the HBM/DMA traffic for activations + weights,
    # which is what bounds the input (SP) DMA ring. PSUM accumulation is
    # fp32 either way; biases and the elementwise tail stay fp32.
    MDT = {"fp32": mybir.dt.float32, "fp32r": mybir.dt.float32r,
           "bf16": mybir.dt.bfloat16}[mm_dtype]

    xT = nc.dram_tensor("xT", [2 * I, BC], MDT, kind="ExternalInput")
    hT = nc.dram_tensor("hT", [H, BC], MDT, kind="ExternalInput")
    sc = nc.dram_tensor("sc", [NBLK, 1, NB], FP32, kind="ExternalInput")
    wg = nc.dram_tensor("wg", [128, 24 * 128], MDT, kind="ExternalInput")
    wh = nc.dram_tensor("wh", [128, 8 * 128], MDT, kind="ExternalInput")
    bg = nc.dram_tensor("bg", [128, 4], FP32, kind="ExternalInput")
    bh = nc.dram_tensor("bh", [128, 2], FP32, kind="ExternalInput")
    outT = nc.dram_tensor("outT", [H, BC], FP32, kind="ExternalOutput")

    # [blk, partition, k-chunk, col] — DMA at `group`-block granularity
    GNB = group * NB
    xTr = xT.rearrange("(k p) (b n) -> b p k n", p=128, n=GNB)
    hTr = hT.rearrange("(k p) (b n) -> b p k n", p=128, n=GNB)
    scr = sc.rearrange("b o n -> b o n") if group == 1 else \
        sc.rearrange("(g j) o n -> g o (j n)", j=group)
    outTr = outT.rearrange("(m p) (b n) -> b p m n", p=128, n=GNB)

    with tile.TileContext(nc) as tc:
        with (
            tc.tile_pool(name="const", bufs=1) as cpool,
            tc.tile_pool(name="xin", bufs=in_bufs) as xpool,
            tc.tile_pool(name="hin", bufs=in_bufs) as hpool,
            tc.tile_pool(name="sin", bufs=in_bufs) as spool,
            tc.tile_pool(name="gates", bufs=work_bufs) as gpool,
            tc.tile_pool(name="work", bufs=work_bufs) as wpool,
            tc.tile_pool(name="outp", bufs=work_bufs) as opool,
            tc.tile_pool(name="psg", bufs=pg_bufs, space=bass.MemorySpace.PSUM) as pgpool,
            tc.tile_pool(name="psh", bufs=ph_bufs, space=bass.MemorySpace.PSUM) as phpool,
        ):
            # Gate weights split per gate-half so the first gate chain only
            # waits on its own 300 KB slice, not the full 2.1 MB weight load.
            wg_sb = cpool.tile([128, 24 * 128], MDT)
            for gi in range(4):
                nc.sync.dma_start(wg_sb[:, gi * 768:(gi + 1) * 768],
                                  wg[:, gi * 768:(gi + 1) * 768])
            bg_sb = cpool.tile([128, 4], FP32)
            nc.sync.dma_start(bg_sb[:], bg[:])
            wh_sb = cpool.tile([128, 8 * 128], MDT)
            nc.sync.dma_start(wh_sb[:], wh[:])
            bh_sb = cpool.tile([128, 2], FP32)
            nc.sync.dma_start(bh_sb[:], bh[:])

            def load_group(g):
                """DMA the inputs for blocks [g*group, (g+1)*group) in one
                burst each, plus the group-wide output staging tile."""
                xt = xpool.tile([128, 4, GNB], MDT, tag="xt")
                if split_loads:
                    # two half-loads: the gate chains only wait on the half
                    # they read next, hiding more DMA latency inside a block
                    nc.sync.dma_start(xt[:, 0:2, :], xTr[g][:, 0:2, :])
                    nc.sync.dma_start(xt[:, 2:4, :], xTr[g][:, 2:4, :])
                else:
                    nc.sync.dma_start(xt[:], xTr[g])
                ht = hpool.tile([128, 2, GNB], MDT, tag="ht")
                nc.sync.dma_start(ht[:], hTr[g])
                srow = spool.tile([1, GNB], FP32, tag="srow")
                nc.sync.dma_start(srow[:], scr[g])
                sbc = spool.tile([128, 2, GNB], FP32, tag="sbc")
                nc.gpsimd.partition_broadcast(sbc[:, 0, :], srow[:])
                nc.gpsimd.partition_broadcast(sbc[:, 1, :], srow[:])
                og = opool.tile([128, 2, GNB], FP32, tag="o")
                return dict(g=g, xt=xt, ht=ht, sbc=sbc, og=og)

            def emit_gates(grp, j):
                """Gate matmuls + sigmoids + r*h for sub-block j of a group."""
                b = grp["g"] * group + j
                js = slice(j * NB, (j + 1) * NB)
                xt = grp["xt"][:, :, js]
                ht = grp["ht"][:, :, js]

                if psum_fine:
                    pgs = [pgpool.tile([128, NB], FP32, tag="pg", name=f"pg{b}_{i}") for i in range(4)]
                else:
                    pg_r = pgpool.tile([128, 2, NB], FP32, tag="pg")
                    pg_u = pgpool.tile([128, 2, NB], FP32, tag="pg")
                    pgs = [pg_r[:, 0, :], pg_r[:, 1, :], pg_u[:, 0, :], pg_u[:, 1, :]]
                for gi in range(4):  # r0, r1, u0, u1
                    dst = pgs[gi][:] if psum_fine else pgs[gi]
                    for k in range(6):
                        act = xt[:, k, :] if k < 4 else ht[:, k - 4, :]
                        c = gi * 6 + k
                        nc.tensor.matmul(
                            dst,
                            wg_sb[:, c * 128:(c + 1) * 128],
                            act,
                            start=(k == 0),
                            stop=(k == 5),
                        )
                r = gpool.tile([128, 2, NB], MDT, tag="r")
                u = gpool.tile([128, 2, NB], FP32, tag="u")
                for m in range(2):
                    nc.scalar.activation(
                        r[:, m, :], pgs[m][:] if psum_fine else pgs[m],
                        AF.Sigmoid, bias=bg_sb[:, m:m + 1]
                    )
                    nc.scalar.activation(
                        u[:, m, :], pgs[2 + m][:] if psum_fine else pgs[2 + m],
                        AF.Sigmoid, bias=bg_sb[:, 2 + m:3 + m]
                    )
                rh = wpool.tile([128, 2, NB], MDT, tag="rh")
                nc.vector.tensor_mul(rh[:], r[:], ht)
                # e2 = score*u and A = h*e2 only depend on the gate phase, so
                # they run here, off the post-tanh critical tail.
                e2 = wpool.tile([128, 2, NB], FP32, tag="e2")
                nc.vector.tensor_mul(e2[:], u[:], grp["sbc"][:, :, js])
                A = wpool.tile([128, 2, NB], FP32, tag="A")
                nc.vector.tensor_mul(A[:], ht, e2[:])
                return dict(b=b, j=j, grp=grp, xt=xt, rh=rh, e2=e2, A=A)

            def emit_h(st):
                """h_hat matmul + tanh + final combine + store for block b."""
                b = st["b"]
                if psum_fine:
                    phs = [phpool.tile([128, NB], FP32, tag="ph", name=f"ph{b}_{i}") for i in range(2)]
                else:
                    ph = phpool.tile([128, 2, NB], FP32, tag="ph")
                    phs = [ph[:, 0, :], ph[:, 1, :]]
                for m in range(2):
                    for k in range(4):
                        act = st["xt"][:, k] if k < 2 else st["rh"][:, k - 2, :]
                        c = m * 4 + k
                        nc.tensor.matmul(
                            phs[m][:] if psum_fine else phs[m],
                            wh_sb[:, c * 128:(c + 1) * 128],
                            act,
                            start=(k == 0),
                            stop=(k == 3),
                        )
                hhat = wpool.tile([128, 2, NB], FP32, tag="hhat")
                for m in range(2):
                    nc.scalar.activation(
                        hhat[:, m, :], phs[m][:] if psum_fine else phs[m],
                        AF.Tanh, bias=bh_sb[:, m:m + 1]
                    )
                # out = A - (e2-1)*hh  ==  hh + e2*(h - hh), with A = h*e2
                C = wpool.tile([128, 2, NB], FP32, tag="C")
                nc.vector.scalar_tensor_tensor(
                    C[:], st["e2"][:], 1.0, hhat[:],
                    op0=mybir.AluOpType.subtract, op1=mybir.AluOpType.mult,
                )
                j = st["j"]
                og = st["grp"]["og"]
                nc.vector.tensor_sub(og[:, :, j * NB:(j + 1) * NB],
                                     st["A"][:], C[:])
                if j == group - 1:
                    # store on the ACT HWDGE ring so it doesn't queue behind
                    # the input loads on the SP ring
                    out_eng = nc.scalar if out_queue == "scalar" else nc.sync
                    out_eng.dma_start(outTr[st["grp"]["g"]], og[:])

            # Software-pipelined emission: block b's h-chain is emitted after
            # block b+1's gate matmuls so the PE never waits on the r*h
            # elementwise product. reps>1 repeats the whole pass (same
            # output) — used only for slope-based timing in bench.py.
            def emit_pass():
                prev = None
                for _rep in range(reps):
                    for g in range(NBLK // group):
                        grp = load_group(g)
                        for j in range(group):
                            st = emit_gates(grp, j)
                            if prev is not None:
                                emit_h(prev)
                            prev = st
                emit_h(prev)

            if loop is None:
                emit_pass()
            else:
                # bench-only: repeat the whole pass `loop` times inside one
                # NEFF execution for slope-based timing.
                with tc.For_i(0, loop, 1):
                    emit_pass()

    nc.compile()
    return nc


def _get_nc():
    if "nc" not in _NC_CACHE:
        _NC_CACHE["nc"] = _build_nc(mm_dtype=MM_DTYPE)
    return _NC_CACHE["nc"]


def _pack_weights(W_r, W_u, W_h, b_r, b_u, b_h):
    wg = np.empty((128, 24 * 128), np.float32)
    for gi in range(4):
        W = W_r if gi < 2 else W_u
        m = gi % 2
        for k in range(6):
            c = gi * 6 + k
            wg[:, c * 128:(c + 1) * 128] = W[m * 128:(m + 1) * 128,
                                             k * 128:(k + 1) * 128].T
    wh = np.empty((128, 8 * 128), np.float32)
    for m in range(2):
        for k in range(4):
            c = m * 4 + k
            wh[:, c * 128:(c + 1) * 128] = W_h[m * 128:(m + 1) * 128,
                                               k * 128:(k + 1) * 128].T
    bg = np.stack([b_r[:128], b_r[128:], b_u[:128], b_u[128:]], axis=1)
    bh = np.stack([b_h[:128], b_h[128:]], axis=1)
    return (np.ascontiguousarray(wg), np.ascontiguousarray(wh),
            np.ascontiguousarray(bg), np.ascontiguousarray(bh))


def _make_in_maps(inputs, h_prev, attention_score, W_r, b_r, W_u, b_u, W_h, b_h):
    inputs = np.asarray(inputs, np.float32)
    h_prev = np.asarray(h_prev, np.float32)
    attention_score = np.asarray(attention_score, np.float32)
    wg, wh, bg, bh = _pack_weights(
        np.asarray(W_r, np.float32), np.asarray(W_u, np.float32),
        np.asarray(W_h, np.float32), np.asarray(b_r, np.float32),
        np.asarray(b_u, np.float32), np.asarray(b_h, np.float32),
    )
    mdt = _HOST_MDT[MM_DTYPE]
    wg = wg.astype(mdt)
    wh = wh.astype(mdt)
    in_maps = []
    for c in range(NCORES):
        sl = slice(c * BC, (c + 1) * BC)
        in_maps.append({
            "xT": np.ascontiguousarray(inputs[sl].T).astype(mdt),
            "hT": np.ascontiguousarray(h_prev[sl].T).astype(mdt),
            "sc": np.ascontiguousarray(attention_score[sl].reshape(NBLK, 1, NB)),
            "wg": wg, "wh": wh, "bg": bg, "bh": bh,
        })
    return in_maps


def _run(in_maps, trace=False, **kwargs):
    try:
        return run_bass_kernel_spmd(
            _get_nc(), in_maps, core_ids=list(range(NCORES)), trace=trace, **kwargs
        )
    except ModuleNotFoundError:
        # A global BASS_TRACE=1 enables the NTFF trace path, which needs
        # antenv.axon_hooks; on images without it, retry untraced. The env
        # override is scoped and restored so other users of the process are
        # unaffected.
        had = os.environ.get("BASS_NEVER_TRACE")
        os.environ["BASS_NEVER_TRACE"] = "1"
        try:
            return run_bass_kernel_spmd(
                _get_nc(), in_maps, core_ids=list(range(NCORES)), trace=False,
                **kwargs
            )
        finally:
            if had is None:
                del os.environ["BASS_NEVER_TRACE"]
            else:
                os.environ["BASS_NEVER_TRACE"] = had


def _gather(results):
    out = np.empty((B, H), np.float32)
    for c in range(NCORES):
        out[c * BC:(c + 1) * BC] = results[c]["outT"].T
    return out


def kernel(**inputs):
    res = _run(_make_in_maps(**inputs), trace=False)
    return _gather(res.results)



# revision 28
# speedup vs baseline: 1.0446x; 1.0325x over previous
"""Trainium2 Bass kernel for a debiased GRU cell.

Computation (per batch row):
    r   = sigmoid(W_r @ [x; h] + b_r)
    u   = sigmoid(W_u @ [x; h] + b_u)
    hh  = tanh(W_h @ [x_int; r*h] + b_h)
    s   = score * u
    out = (1 - s) * hh + s * h

Strategy: data-parallel over 8 cores (8192 rows each). On-chip layout is
feature-major ([H, batch]) so that
  - activations never need an on-chip transpose (host supplies x.T / h.T),
  - gate biases fuse into the ACT engine's per-partition bias operand,
  - matmuls run with full K=128 / M=128 / N=512 tiles (PE at peak rate).
The only broadcast needed (attention score along partitions) runs on the
otherwise-idle GPSIMD engine. Output is produced as out.T and un-transposed
on the host.
"""

import os

import ml_dtypes
import numpy as np

import concourse.bacc as bacc
import concourse.bass as bass
import concourse.mybir as mybir
import concourse.tile as tile
from concourse.bass_utils import run_bass_kernel_spmd

B = 65536
I = 256
H = 256
NCORES = 8
BC = B // NCORES  # rows per core
NB = 512          # batch columns per block (max fp32 matmul free dim)
NBLK = BC // NB   # 16
FP32 = mybir.dt.float32
AF = mybir.ActivationFunctionType

# Host-side dtype for matmul operands; must match _build_nc's mm_dtype.
MM_DTYPE = "bf16"
_HOST_MDT = {"fp32": np.float32, "fp32r": np.float32,
             "bf16": ml_dtypes.bfloat16, "fp8": ml_dtypes.bfloat16}
_HOST_FP8 = ml_dtypes.float8_e4m3

_NC_CACHE = {}


def _build_nc(reps=1, loop=None, mm_dtype="bf16",
              pg_bufs=6, ph_bufs=2, in_bufs=4, work_bufs=3, psum_fine=True,
              out_queue="scalar", split_loads=True, group=1, staggered=True):
    nc = bacc.Bacc(
        "TRN2",
        target_bir_lowering=False,
        debug=False,
        enable_asserts=False,
    )

    # Matmul-operand dtype. float32r streams fp32 bits through the PE at
    # full rate (1 cycle/row vs 4 for plain fp32); bf16 runs at the same
    # PE rate but halves the HBM/DMA traffic for activations + weights,
    # which is what bounds the input (SP) DMA ring. PSUM accumulation is
    # fp32 either way; biases stay fp32.
    #
    # "fp8": the GATE matmuls (3/4 of PE work) take float8e4 operands in
    # DoubleRow mode (2 contraction rows/cycle): each gate-half runs as 3
    # K=256 matmuls instead of 6 K=128 ones. The h_hat matmul and all
    # elementwise stay bf16 — only the pre-sigmoid gate accumulations see
    # fp8 rounding, and those pass through sigmoid (gain <= 1/4).
    FP8 = mm_dtype == "fp8"
    MDT = {"fp32": mybir.dt.float32, "fp32r": mybir.dt.float32r,
           "bf16": mybir.dt.bfloat16, "fp8": mybir.dt.bfloat16}[mm_dtype]
    QDT = mybir.dt.float8e4

    # EDT: elementwise dtype. bf16 operands qualify for the DVE 2x/4x
    # packed-16-bit fast paths, which matters because the fp32 elementwise
    # chain (~5 ops/block at ~1.1 us each) nearly rate-matches the PE.
    EDT = MDT if mm_dtype in ("bf16", "fp8") else FP32
    # fp8 mode ships both operand precisions as ONE tensor each so steady
    # blocks need a single input DMA per precision:
    #   xh8 = [x (4 chunks); h (2 chunks)] fp8   -> gate matmuls
    #   xh  = [x_int (2 chunks); h (2 chunks)] bf16 -> h_hat matmul + DVE
    if FP8:
        xT = nc.dram_tensor("xh", [I + H, BC], MDT, kind="ExternalInput")
        xh8 = nc.dram_tensor("xh8", [2 * I + H, BC], QDT, kind="ExternalInput")
    else:
        xT = nc.dram_tensor("xT", [2 * I, BC], MDT, kind="ExternalInput")
        hT = nc.dram_tensor("hT", [H, BC], MDT, kind="ExternalInput")
    sc = nc.dram_tensor("sc", [NBLK, 1, NB], EDT, kind="ExternalInput")
    wg = nc.dram_tensor("wg", [128, 24 * 128], QDT if FP8 else MDT,
                        kind="ExternalInput")
    wh = nc.dram_tensor("wh", [128, 8 * 128], MDT, kind="ExternalInput")
    bg = nc.dram_tensor("bg", [128, 4], FP32, kind="ExternalInput")
    bh = nc.dram_tensor("bh", [128, 2], FP32, kind="ExternalInput")
    outT = nc.dram_tensor("outT", [H, BC], EDT, kind="ExternalOutput")

    # [blk, partition, k-chunk, col] — DMA at `group`-block granularity
    GNB = group * NB
    xTr = xT.rearrange("(k p) (b n) -> b p k n", p=128, n=GNB)
    if FP8:
        xh8r = xh8.rearrange("(k p) (b n) -> b p k n", p=128, n=GNB)
    else:
        hTr = hT.rearrange("(k p) (b n) -> b p k n", p=128, n=GNB)
    scr = sc.rearrange("b o n -> b o n") if group == 1 else \
        sc.rearrange("(g j) o n -> g o (j n)", j=group)
    outTr = outT.rearrange("(m p) (b n) -> b p m n", p=128, n=GNB)

    with tile.TileContext(nc) as tc:
        with (
            tc.tile_pool(name="const", bufs=1) as cpool,
            tc.tile_pool(name="xin", bufs=in_bufs) as xpool,
            tc.tile_pool(name="hin", bufs=in_bufs) as hpool,
            tc.tile_pool(name="sin", bufs=in_bufs) as spool,
            tc.tile_pool(name="gates", bufs=work_bufs) as gpool,
            tc.tile_pool(name="work", bufs=work_bufs) as wpool,
            tc.tile_pool(name="outp", bufs=work_bufs) as opool,
            tc.tile_pool(name="psg", bufs=pg_bufs, space=bass.MemorySpace.PSUM) as pgpool,
            tc.tile_pool(name="psh", bufs=ph_bufs, space=bass.MemorySpace.PSUM) as phpool,
        ):
            # Gate weights as one tile PER gate-half: tile-granular dep
            # tracking means a single big tile would stall the first
            # Ldweights on the whole weight load. Only wg0 rides the SP
            # ring (ahead of block 0's activations); everything else goes
            # on the scalar/ACT HWDGE ring, which is idle at startup.
            if FP8:
                # [gate-half][p, k-pair, pair-member, M] DoubleRow stationary
                wgr = wg.rearrange("p (g a b m) -> g p a b m", g=4, a=3, b=2)
            wg_sb = []
            for gi in range(4):
                if FP8:
                    w = cpool.tile([128, 3, 2, 128], QDT, name=f"wg{gi}")
                    src = wgr[gi]
                else:
                    w = cpool.tile([128, 6 * 128], MDT, name=f"wg{gi}")
                    src = wg[:, gi * 768:(gi + 1) * 768]
                eng = nc.sync if gi == 0 else nc.scalar
                eng.dma_start(w[:], src)
                wg_sb.append(w)
            bg_sb = cpool.tile([128, 4], FP32)
            nc.scalar.dma_start(bg_sb[:], bg[:])
            wh_sb = cpool.tile([128, 8 * 128], MDT)
            nc.scalar.dma_start(wh_sb[:], wh[:])
            bh_sb = cpool.tile([128, 2], FP32)
            nc.scalar.dma_start(bh_sb[:], bh[:])

            def load_group(g, fine=False):
                """DMA the inputs for blocks [g*group, (g+1)*group) in one
                burst each, plus the group-wide output staging tile.

                fine=True (pipeline-fill path, block 0 only): each k-chunk
                gets its OWN tile + DMA. Dep tracking is per-tile, so with
                one big tile the first matmul would wait for the whole x
                load; with chunk tiles it starts after 1/4 of it.
                """
                xp = hp = None
                if FP8:
                    # fp8 gate operands, as two k-pair tiles for x so the
                    # first DoubleRow matmul starts after half the x load
                    xp = []
                    for k2 in range(2):
                        t = xpool.tile([128, 2, GNB], QDT, tag=f"x8{k2}")
                        nc.sync.dma_start(t[:], xT8r[g][:, 2 * k2:2 * k2 + 2, :])
                        xp.append(t)
                    hp = xpool.tile([128, 2, GNB], QDT, tag="h8")
                    nc.sync.dma_start(hp[:], hT8r[g])
                    # bf16 x_int chunks feed the h_hat matmul only
                    xt = xpool.tile([128, 2, GNB], MDT, tag="xt")
                    nc.sync.dma_start(xt[:], xTr[g])
                    ht2 = hpool.tile([128, 2, GNB], MDT, tag="ht")
                    nc.sync.dma_start(ht2[:], hTr[g])
                    xk = [xt[:, k, :] for k in range(2)]
                    hk = [ht2[:, k, :] for k in range(2)]
                elif fine:
                    xk = []
                    for k in range(4):
                        t = xpool.tile([128, GNB], MDT, tag=f"xf{k}")
                        nc.sync.dma_start(t[:], xTr[g][:, k, :])
                        xk.append(t[:])
                    hk = []
                    for k in range(2):
                        t = hpool.tile([128, GNB], MDT, tag=f"hf{k}")
                        nc.sync.dma_start(t[:], hTr[g][:, k, :])
                        hk.append(t[:])
                    ht2 = None
                else:
                    xt = xpool.tile([128, 4, GNB], MDT, tag="xt")
                    if split_loads:
                        nc.sync.dma_start(xt[:, 0:2, :], xTr[g][:, 0:2, :])
                        nc.sync.dma_start(xt[:, 2:4, :], xTr[g][:, 2:4, :])
                    else:
                        nc.sync.dma_start(xt[:], xTr[g])
                    ht2 = hpool.tile([128, 2, GNB], MDT, tag="ht")
                    nc.sync.dma_start(ht2[:], hTr[g])
                    xk = [xt[:, k, :] for k in range(4)]
                    hk = [ht2[:, k, :] for k in range(2)]
                srow = spool.tile([1, GNB], EDT, tag="srow")
                nc.sync.dma_start(srow[:], scr[g])
                sbc = spool.tile([128, 2, GNB], EDT, tag="sbc")
                nc.gpsimd.partition_broadcast(sbc[:, 0, :], srow[:])
                nc.gpsimd.partition_broadcast(sbc[:, 1, :], srow[:])
                og = opool.tile([128, 2, GNB], EDT, tag="o")
                return dict(g=g, xk=xk, hk=hk, ht=ht2, sbc=sbc, og=og,
                            xp=xp, hp=hp)

            def emit_gates(grp, j):
                """Gate matmuls + sigmoids + r*h for sub-block j of a group."""
                b = grp["g"] * group + j
                js = slice(j * NB, (j + 1) * NB)
                xk = [a[:, js] for a in grp["xk"]]
                hk = [a[:, js] for a in grp["hk"]]

                if psum_fine:
                    pgs = [pgpool.tile([128, NB], FP32, tag="pg", name=f"pg{b}_{i}") for i in range(4)]
                else:
                    pg_r = pgpool.tile([128, 2, NB], FP32, tag="pg")
                    pg_u = pgpool.tile([128, 2, NB], FP32, tag="pg")
                    pgs = [pg_r[:, 0, :], pg_r[:, 1, :], pg_u[:, 0, :], pg_u[:, 1, :]]
                if FP8:
                    xpj = [a[:, :, js] for a in grp["xp"]]
                    hpj = grp["hp"][:, :, js]
                for gi in range(4):  # r0, r1, u0, u1
                    dst = pgs[gi][:] if psum_fine else pgs[gi]
                    if FP8:
                        for k2 in range(3):  # k-pairs (x01, x23, h01)
                            rhs = xpj[k2] if k2 < 2 else hpj
                            nc.tensor.matmul(
                                dst,
                                wg_sb[gi][:, k2],
                                rhs,
                                start=(k2 == 0),
                                stop=(k2 == 2),
                                perf_mode=mybir.MatmulPerfMode.DoubleRow,
                            )
                    else:
                        for k in range(6):
                            act = xk[k] if k < 4 else hk[k - 4]
                            nc.tensor.matmul(
                                dst,
                                wg_sb[gi][:, k * 128:(k + 1) * 128],
                                act,
                                start=(k == 0),
                                stop=(k == 5),
                            )
                r = gpool.tile([128, 2, NB], MDT, tag="r")
                u = gpool.tile([128, 2, NB], EDT, tag="u")
                for m in range(2):
                    nc.scalar.activation(
                        r[:, m, :], pgs[m][:] if psum_fine else pgs[m],
                        AF.Sigmoid, bias=bg_sb[:, m:m + 1]
                    )
                    nc.scalar.activation(
                        u[:, m, :], pgs[2 + m][:] if psum_fine else pgs[2 + m],
                        AF.Sigmoid, bias=bg_sb[:, 2 + m:3 + m]
                    )
                rh = wpool.tile([128, 2, NB], MDT, tag="rh")
                if grp["ht"] is not None:
                    nc.vector.tensor_mul(rh[:], r[:], grp["ht"][:, :, js])
                else:
                    for m in range(2):
                        nc.vector.tensor_mul(rh[:, m, :], r[:, m, :], hk[m])
                # e2 = score*u and A = h*e2 only depend on the gate phase, so
                # they run here, off the post-tanh critical tail.
                e2 = wpool.tile([128, 2, NB], EDT, tag="e2")
                nc.vector.tensor_mul(e2[:], u[:], grp["sbc"][:, :, js])
                A = wpool.tile([128, 2, NB], EDT, tag="A")
                if grp["ht"] is not None:
                    nc.vector.tensor_mul(A[:], grp["ht"][:, :, js], e2[:])
                else:
                    for m in range(2):
                        nc.vector.tensor_mul(A[:, m, :], hk[m], e2[:, m, :])
                return dict(b=b, j=j, grp=grp, xk=xk, rh=rh, e2=e2, A=A)

            def emit_h(st, parts=1):
                """h_hat matmul + tanh + final combine + store for block b.

                parts>1 splits the chain into column sub-chunks so the
                serial matmul->tanh->combine->store dependency chain at the
                pipeline drain is ~1/parts as long (each chunk's tail
                overlaps the next chunk's matmuls). Used for the last block
                of a pass; per-part output stores go out immediately.
                """
                b = st["b"]
                j = st["j"]
                og = st["grp"]["og"]
                NP = NB // parts
                out_eng = nc.scalar if out_queue == "scalar" else nc.sync
                # drain-phase stores ride the SP ring: the input ring is
                # idle at the end of a pass, the ACT ring is not
                tail_eng = nc.sync
                if parts > 1 and j > 0:
                    # flush the group's earlier blocks (normally stored in
                    # one group-wide DMA at j == group-1)
                    lead = slice(0, j * NB)
                    tail_eng.dma_start(outTr[st["grp"]["g"]][:, :, lead],
                                       og[:, :, lead])
                for p in range(parts):
                    cs = slice(p * NP, (p + 1) * NP)
                    if psum_fine or parts > 1:
                        phs = [phpool.tile([128, NP], FP32, tag="ph",
                                           name=f"ph{b}_{p}_{i}") for i in range(2)]
                    else:
                        ph = phpool.tile([128, 2, NB], FP32, tag="ph")
                        phs = [ph[:, 0, :], ph[:, 1, :]]
                    for m in range(2):
                        for k in range(4):
                            act = (st["xk"][k][:, cs] if k < 2
                                   else st["rh"][:, k - 2, cs])
                            c = m * 4 + k
                            nc.tensor.matmul(
                                phs[m][:] if (psum_fine or parts > 1) else phs[m],
                                wh_sb[:, c * 128:(c + 1) * 128],
                                act,
                                start=(k == 0),
                                stop=(k == 3),
                            )
                    hhat = wpool.tile([128, 2, NP], EDT, tag=f"hhat_p{parts}")
                    for m in range(2):
                        nc.scalar.activation(
                            hhat[:, m, :],
                            phs[m][:] if (psum_fine or parts > 1) else phs[m],
                            AF.Tanh, bias=bh_sb[:, m:m + 1]
                        )
                    # out = A - (e2-1)*hh  ==  hh + e2*(h - hh), with A = h*e2
                    C = wpool.tile([128, 2, NP], EDT, tag=f"C_p{parts}")
                    nc.vector.scalar_tensor_tensor(
                        C[:], st["e2"][:, :, cs], 1.0, hhat[:],
                        op0=mybir.AluOpType.subtract, op1=mybir.AluOpType.mult,
                    )
                    ocs = slice(j * NB + p * NP, j * NB + (p + 1) * NP)
                    nc.vector.tensor_sub(og[:, :, ocs], st["A"][:, :, cs], C[:])
                    if parts > 1:
                        # store each finished chunk immediately so the last
                        # store isn't serialized behind the whole block
                        tail_eng.dma_start(outTr[st["grp"]["g"]][:, :, ocs],
                                           og[:, :, ocs])
                if parts == 1 and j == group - 1:
                    # store on the ACT HWDGE ring so it doesn't queue behind
                    # the input loads on the SP ring
                    out_eng.dma_start(outTr[st["grp"]["g"]], og[:])

            # Software-pipelined emission: block b's h-chain is emitted after
            # block b+1's gate matmuls so the PE never waits on the r*h
            # elementwise product. reps>1 repeats the whole pass (same
            # output) — used only for slope-based timing in bench.py.
            def emit_pass():
                prev = None
                for _rep in range(reps):
                    for g in range(NBLK // group):
                        grp = load_group(g, fine=(g == 0 and _rep == 0))
                        for j in range(group):
                            st = emit_gates(grp, j)
                            if prev is not None:
                                emit_h(prev)
                            prev = st
                emit_h(prev, parts=4)

            if loop is None:
                emit_pass()
            else:
                # bench-only: repeat the whole pass `loop` times inside one
                # NEFF execution for slope-based timing. staggered_reset
                # removes the all-engine barrier + sem reset on the loop
                # back-edge (resets are staggered across 4 body stages), so
                # consecutive passes pipeline like a real streaming workload.
                with tc.For_i(0, loop, 1, staggered_reset=staggered):
                    emit_pass()

    nc.compile()
    return nc


def _get_nc():
    if "nc" not in _NC_CACHE:
        _NC_CACHE["nc"] = _build_nc(mm_dtype=MM_DTYPE)
    return _NC_CACHE["nc"]


def _pack_weights(W_r, W_u, W_h, b_r, b_u, b_h):
    wg = np.empty((128, 24 * 128), np.float32)
    for gi in range(4):
        W = W_r if gi < 2 else W_u
        m = gi % 2
        for k in range(6):
            c = gi * 6 + k
            wg[:, c * 128:(c + 1) * 128] = W[m * 128:(m + 1) * 128,
                                             k * 128:(k + 1) * 128].T
    wh = np.empty((128, 8 * 128), np.float32)
    for m in range(2):
        for k in range(4):
            c = m * 4 + k
            wh[:, c * 128:(c + 1) * 128] = W_h[m * 128:(m + 1) * 128,
                                               k * 128:(k + 1) * 128].T
    bg = np.stack([b_r[:128], b_r[128:], b_u[:128], b_u[128:]], axis=1)
    bh = np.stack([b_h[:128], b_h[128:]], axis=1)
    return (np.ascontiguousarray(wg), np.ascontiguousarray(wh),
            np.ascontiguousarray(bg), np.ascontiguousarray(bh))


def _make_in_maps(inputs, h_prev, attention_score, W_r, b_r, W_u, b_u, W_h, b_h):
    inputs = np.asarray(inputs, np.float32)
    h_prev = np.asarray(h_prev, np.float32)
    attention_score = np.asarray(attention_score, np.float32)
    wg, wh, bg, bh = _pack_weights(
        np.asarray(W_r, np.float32), np.asarray(W_u, np.float32),
        np.asarray(W_h, np.float32), np.asarray(b_r, np.float32),
        np.asarray(b_u, np.float32), np.asarray(b_h, np.float32),
    )
    mdt = _HOST_MDT[MM_DTYPE]
    fp8 = MM_DTYPE == "fp8"
    wg = wg.astype(_HOST_FP8 if fp8 else mdt)
    wh = wh.astype(mdt)
    in_maps = []
    for c in range(NCORES):
        sl = slice(c * BC, (c + 1) * BC)
        xTc = np.ascontiguousarray(inputs[sl].T)
        hTc = np.ascontiguousarray(h_prev[sl].T)
        m = {
            "xT": xTc[:I].astype(mdt) if fp8 else xTc.astype(mdt),
            "hT": hTc.astype(mdt),
            "sc": np.ascontiguousarray(
                attention_score[sl].reshape(NBLK, 1, NB)).astype(mdt),
            "wg": wg, "wh": wh, "bg": bg, "bh": bh,
        }
        if fp8:
            m["xT8"] = xTc.astype(_HOST_FP8)
            m["hT8"] = hTc.astype(_HOST_FP8)
        in_maps.append(m)
    return in_maps


def _run(in_maps, trace=False, **kwargs):
    try:
        return run_bass_kernel_spmd(
            _get_nc(), in_maps, core_ids=list(range(NCORES)), trace=trace, **kwargs
        )
    except ModuleNotFoundError:
        # A global BASS_TRACE=1 enables the NTFF trace path, which needs
        # antenv.axon_hooks; on images without it, retry untraced. The env
        # override is scoped and restored so other users of the process are
        # unaffected.
        had = os.environ.get("BASS_NEVER_TRACE")
        os.environ["BASS_NEVER_TRACE"] = "1"
        try:
            return run_bass_kernel_spmd(
                _get_nc(), in_maps, core_ids=list(range(NCORES)), trace=False,
                **kwargs
            )
        finally:
            if had is None:
                del os.environ["BASS_NEVER_TRACE"]
            else:
                os.environ["BASS_NEVER_TRACE"] = had


def _gather(results):
    out = np.empty((B, H), np.float32)
    for c in range(NCORES):
        out[c * BC:(c + 1) * BC] = results[c]["outT"].T.astype(np.float32)
    return out


def kernel(**inputs):
    res = _run(_make_in_maps(**inputs), trace=False)
    return _gather(res.results)

